# revision 5
# baseline (speedup 1.0000x reference)
"""Causal self-attention (B=4, T=2048, C=1024, H=16, HD=64) on 8 trn2 cores.

Sharding: tensor-parallel over 2 head groups x data-parallel over 4 batches.
Core i handles batch (i % 4), head group (i // 4) (8 heads each).
Each core computes q/k/v projections for its head slice, causal attention,
and a partial output projection; the host sums the two head-group partials
per batch and transposes back.

All matmuls run in bf16 with fp32 PSUM accumulation.  Layouts are kept
"transposed" ([feature, token]) end-to-end so no on-chip transposes are
needed:
  qT,kT = W.T^T @ xT          [d_local=512, T]
  V     = x @ Wv.T            [T, d_local]  (+ interleaved ones column)
  ST    = kT_h^T-form product [k, q] tiles, softmax over k (partition dim)
  oT    = [V|1]^T @ exp(ST)   [65, q]  (row 64 = softmax denominator)
  yT    = woT^T @ oT          [C, T]   (partial; host sums head groups)
"""

import json as _json

import numpy as np
import ml_dtypes

import concourse.bass as bass
import concourse.mybir as mybir
import concourse.bass2jax as _b2j
import concourse.bass_utils as _bu
from concourse import tile
from concourse.bass_utils import compile_bir_kernel as _orig_compile_bir_kernel

# ---------------------------------------------------------------------------
# Workaround: the neuronxcc walrus in this container rejects more than one
# sync-wait command per instruction ("Too many sync wait commands").  Tile
# routinely emits several waits on one instruction (and many on the kernel
# tail drain).  Rewrite the BIR before walrus: hoist extra waits onto NoOp
# carrier instructions inserted just before the over-limit instruction on the
# same engine stream.
_BIR_MAXW = 1
import os as _os
_BIR_MAXW_COMPUTE = int(_os.environ.get("BIR_MAXW_COMPUTE", "1"))


def _split_bir_waits(bir_str):
    j = _json.loads(bir_str)
    ctr = 0
    for fn in j.get("functions", []):
        for blk in fn.get("blocks", []):
            new_insts = []
            for ins in blk["instructions"]:
                si = ins.get("sync_info")
                waits = (si or {}).get("on_wait") or []
                maxw = _BIR_MAXW if ins["opcode"] in ("Drain", "NoOp") else _BIR_MAXW_COMPUTE
                if len(waits) > maxw:
                    extra, keep = waits[:-maxw], waits[-maxw:]
                    for i in range(0, len(extra), _BIR_MAXW):
                        ctr += 1
                        new_insts.append(
                            {
                                "debug": ins.get("debug", 0),
                                "engine": ins["engine"],
                                "ins": [],
                                "name": f"{ins['name']}-sw{ctr}",
                                "opcode": "NoOp",
                                "outs": [],
                                "sync_info": {
                                    "on_update": [],
                                    "on_wait": extra[i : i + _BIR_MAXW],
                                },
                            }
                        )
                    si["on_wait"] = keep
                new_insts.append(ins)
            blk["instructions"] = new_insts
    return _json.dumps(j).encode()


def _patched_compile_bir_kernel(ant_bir_str, *args, **kwargs):
    ant_bir_str = _split_bir_waits(ant_bir_str)
    return _orig_compile_bir_kernel(ant_bir_str, *args, **kwargs)


_b2j.compile_bir_kernel = _patched_compile_bir_kernel
_bu.compile_bir_kernel = _patched_compile_bir_kernel
# ---------------------------------------------------------------------------

F32 = mybir.dt.float32
BF16 = mybir.dt.bfloat16
BF16_NP = ml_dtypes.bfloat16

N_EMBED = 1024
N_HEADS = 16
B = 4
HD = 64
CHUNK = 512  # q-chunk width (PSUM bank / max matmul N)
SEG = 1024  # exp segment width (2 PSUM banks)


def build_nc(T=2048, n_heads_local=8, causal=True, seg=SEG, st_bufs=3, reps=1, phases=3, av_bufs=3, proj_bufs=2, group=2, sched="j", nv_front=4, pair_exp=True, mask_pool=True):
    """One-core program; SPMD across 8 cores with per-core inputs.

    PSUM rings: st (attention score tiles + bc broadcasts), av (per-head
    output accumulators), proj (q/k/v/y projection accumulators).  Giving
    projections their own ring lets the Tile scheduler hoist projection
    matmuls into the PE gaps of the ACT-bound attention inner loop, keeping
    the tensor engine continuously busy (HAM stays at K=8/8).
    """
    C = N_EMBED
    HL = n_heads_local
    DL = HL * HD  # local head dim total (512)
    NC_C = C // 128  # 8 c-tiles
    NQK = 2 * DL // 128  # 8 d'-tiles for q|k
    NT_T = T // 128  # t-subtiles for V
    NJ = T // CHUNK  # q chunks
    NKT = T // 128  # k tiles
    VW = HL * (HD + 1)  # vp tile width (520)
    if pair_exp:
        # stp tiles span 2 PSUM banks: 2*2 + av 2 + proj 2 = 8 banks
        st_bufs = min(st_bufs, 2)
        av_bufs = min(av_bufs, 8 - 2 * st_bufs - proj_bufs)

    nc = bass.Bass()
    xt_d = nc.dram_tensor("xt", [C, T], BF16, kind="ExternalInput")
    wqk_d = nc.dram_tensor("wqk", [C, 2 * DL], BF16, kind="ExternalInput")
    wv_d = nc.dram_tensor("wv", [C, DL], BF16, kind="ExternalInput")
    wot_d = nc.dram_tensor("wot", [DL, C], BF16, kind="ExternalInput")
    yt_d = nc.dram_tensor("yt", [C, T], F32, kind="ExternalOutput")

    with tile.TileContext(nc) as tc:
        with (
            tc.tile_pool(name="px", bufs=NC_C) as px,
            tc.tile_pool(name="pwqk", bufs=NC_C) as pwqk,
            tc.tile_pool(name="pwv", bufs=NC_C) as pwv,
            tc.tile_pool(name="pwot", bufs=DL // 128) as pwot,
            tc.tile_pool(name="pqk", bufs=NQK) as pqk,
            tc.tile_pool(name="pvp", bufs=NT_T) as pvp,
            tc.tile_pool(name="pe", bufs=8 if pair_exp else 16) as pe,
            tc.tile_pool(name="pot", bufs=DL // 128) as pot,
            tc.tile_pool(name="pr", bufs=8) as pr,
            tc.tile_pool(name="pbc", bufs=4) as pbc,
            tc.tile_pool(name="pysb", bufs=6) as pysb,
            tc.tile_pool(name="pones", bufs=1) as pones,
            tc.tile_pool(name="psum", bufs=2, space="PSUM") as psum,
        ):
          import contextlib
          loop_ctx = tc.For_i(0, reps, 1) if reps > 1 else contextlib.nullcontext()
          with loop_ctx:
            # --- load inputs -------------------------------------------------
            xts = []
            wqks = []
            wvs = []
            # input loads ride the ACT HWDGE ring: the SP ring carries the y
            # stores, so next rep's loads would otherwise queue behind them
            for c in range(NC_C):
                xt = px.tile([128, T], BF16, tag="xt")
                nc.scalar.dma_start(xt[:], xt_d[c * 128 : (c + 1) * 128, :])
                xts.append(xt)
                wqk = pwqk.tile([128, 2 * DL], BF16, tag="wqk")
                nc.scalar.dma_start(wqk[:], wqk_d[c * 128 : (c + 1) * 128, :])
                wqks.append(wqk)
            if phases >= 2:
                for c in range(NC_C):
                    wv = pwv.tile([128, DL], BF16, tag="wv")
                    nc.scalar.dma_start(wv[:], wv_d[c * 128 : (c + 1) * 128, :])
                    wvs.append(wv)
            wots = []
            if phases >= 3:
                for d in range(DL // 128):
                    wot = pwot.tile([128, C], BF16, tag="wot")
                    nc.scalar.dma_start(wot[:], wot_d[d * 128 : (d + 1) * 128, :])
                    wots.append(wot)
            ones = pones.tile([128, 64], BF16, tag="ones")
            nc.gpsimd.memset(ones[:], 1.0)
            masks = []
            id128 = None
            if pair_exp and causal and not mask_pool:
                # identity (bf16) streams causal-mask biases into PSUM; mask m
                # is -1e9 where q < k within diag tile m (also covers the
                # unwritten cols [0, 128m) of trimmed diagonal score tiles)
                id128 = pones.tile([128, 128], BF16, tag="id128")
                nc.gpsimd.memset(id128[:], 1.0)
                nc.gpsimd.affine_select(
                    out=id128[:],
                    in_=id128[:],
                    compare_op=mybir.AluOpType.is_equal,
                    fill=0.0,
                    base=0,
                    pattern=[[-1, 128]],
                    channel_multiplier=1,
                )
                for m in range(CHUNK // 128):
                    mk = pones.tile([128, CHUNK], BF16, tag=f"mask{m}", name=f"mask{m}")
                    nc.gpsimd.memset(mk[:], 0.0)
                    nc.gpsimd.affine_select(
                        out=mk[:],
                        in_=mk[:],
                        compare_op=mybir.AluOpType.is_ge,
                        fill=-1e9,
                        base=-128 * m,
                        pattern=[[1, CHUNK]],
                        channel_multiplier=-1,
                    )
                    masks.append(mk)

            # --- q/k projections (transposed layout), per head pair ---------
            # qk tile dq holds d' rows [dq*128, dq*128+128) of [q | k]T
            def project_qk(dq):
                qk = pqk.tile([128, T], BF16, tag="qk", name=f"qk{dq}")
                for jp in range(NJ):
                    ps = psum.tile([128, CHUNK], F32, tag="proj", bufs=proj_bufs, name=f"qkps{dq}_{jp}")
                    for c in range(NC_C):
                        nc.tensor.matmul(
                            ps[:],
                            wqks[c][:, dq * 128 : (dq + 1) * 128],
                            xts[c][:, jp * CHUNK : (jp + 1) * CHUNK],
                            start=(c == 0),
                            stop=(c == NC_C - 1),
                        )
                    nc.vector.tensor_copy(
                        qk[:, jp * CHUNK : (jp + 1) * CHUNK], ps[:]
                    )
                return qk

            # --- V projection (straight layout, ones col per head) ----------
            # vp tile layout per head h: [V(64) | 1] at h*65
            def project_v(ts):
                vp = pvp.tile([128, VW], BF16, tag="vp", name=f"vp{ts}")
                ps = psum.tile([128, CHUNK], F32, tag="proj", bufs=proj_bufs, name=f"vps{ts}")
                for c in range(NC_C):
                    nc.tensor.matmul(
                        ps[:],
                        xts[c][:, ts * 128 : (ts + 1) * 128],
                        wvs[c][:],
                        start=(c == 0),
                        stop=(c == NC_C - 1),
                    )
                ps3 = ps[:].rearrange("p (h c) -> p h c", c=HD)
                vp3 = vp[:].rearrange("p (h c) -> p h c", c=HD + 1)
                nc.vector.tensor_copy(vp3[:, :, 0:HD], ps3[:, :, :])
                nc.gpsimd.memset(vp3[:, :, HD : HD + 1], 1.0)
                return vp

            qk_tiles = {}
            vps = []
            if sched == "j":
                # all q/k upfront (j-outer attention needs every group at j=0),
                # then the first nv_front V tiles; the rest are emitted later as
                # PE gap fillers
                for dq in range(NQK):
                    qk_tiles[dq] = project_qk(dq)
                if phases >= 2:
                    for ts in range(nv_front):
                        vps.append(project_v(ts))
            else:
                # group 0's q/k first so attention can begin while V projects
                for pp in range(group // 2):
                    qk_tiles[pp] = project_qk(pp)
                    qk_tiles[NQK // 2 + pp] = project_qk(NQK // 2 + pp)
                if phases >= 2:
                    for ts in range(NT_T):
                        vps.append(project_v(ts))

            # --- attention per head -----------------------------------------
            if phases == 1:
                for dq in range(NQK):
                    qkx = qk_tiles.get(dq) or project_qk(dq)
                    nc.sync.dma_start(
                        yt_d[dq * 128 : (dq + 1) * 128, : T // 2].bitcast(BF16),
                        qkx[:],
                    )
                return nc
            ots = [pot.tile([128, T], BF16, tag="ot", name=f"ot{i}") for i in range(DL // 128)]
            kpb = CHUNK // 128  # k-tiles per chunk
            stb = st_bufs
            tmp_bufs = 2 if sched == "g" else HL // 2
            tmps = {
                pp: pot.tile([64, T], BF16, tag="ottmp", bufs=tmp_bufs, name=f"ottmp{pp}")
                for pp in range(HL // 2)
            }

            def normalize(j, av, h):
                # ot[h rows, :] = av[hd rows] * (1/denom) bcast (PE outer
                # product broadcasts the reciprocal row over 64 partitions)
                pp = h // 2
                r = pr.tile([128, CHUNK], BF16, tag="r", name=f"r{h}_{j}")
                with nc.allow_low_precision("bf16 softmax denom (~4e-3 ok)"):
                    nc.vector.reciprocal(r[64:65, :], av[64:65, :])
                bc = psum.tile(
                    [128, CHUNK], F32, tag="proj" if pair_exp else "st",
                    bufs=proj_bufs if pair_exp else stb,
                    name=f"bc{h}_{j}",
                )
                nc.tensor.matmul(
                    bc[0:64, :], ones[64:65, :], r[64:65, :],
                    start=True, stop=True,
                )
                bcs = pbc.tile([128, CHUNK], F32, tag="bc", name=f"bcs{h}_{j}")
                nc.vector.tensor_copy(bcs[0:64, :], bc[0:64, :])
                mul_out = (
                    ots[pp][0:64, j * CHUNK : (j + 1) * CHUNK]
                    if h % 2 == 0
                    else tmps[pp][:, j * CHUNK : (j + 1) * CHUNK]
                )
                nc.vector.tensor_mul(mul_out, av[0:64, :], bcs[0:64, :])
                if h % 2 == 1:
                    # land odd-head rows per chunk so y-proj of chunk j can
                    # start during the last group's later chunks
                    nc.sync.dma_start(
                        ots[pp][64:128, j * CHUNK : (j + 1) * CHUNK],
                        tmps[pp][:, j * CHUNK : (j + 1) * CHUNK],
                    )

            def attend_chunk_pair_v2(heads, qs, ks, j):
                # v2: STs for the two heads of a pair interleave so adjacent
                # PE instructions hit disjoint row groups (rows 0-63 / 64-127)
                # and run concurrently in the array; causal masking moves off
                # PE entirely (Pool affine_select zeroes the exp'd diag
                # region), and exp/AV skip the fully-masked leading columns.
                avs = [
                    psum.tile(
                        [128, CHUNK], F32, tag="av", bufs=av_bufs,
                        name=f"av{h}_{j}",
                    )
                    for h in heads
                ]
                last_kt = kpb * j + kpb - 1 if causal else NKT - 1
                npairs = (last_kt + 1) // 2
                pending = {h: [] for h in heads}

                def flush_avs(hh, h):
                    for e, kts_, d0p in pending[h]:
                        for i, kt in enumerate(kts_):
                            nc.tensor.matmul(
                                avs[hh][0:65, d0p:CHUNK],
                                vps[kt][:, h * (HD + 1) : (h + 1) * (HD + 1)],
                                e[:, i * CHUNK + d0p : (i + 1) * CHUNK],
                                start=(kt == 0),
                                stop=(kt == last_kt),
                            )
                    pending[h] = []

                for kp in range(npairs):
                    kts = (2 * kp, 2 * kp + 1)
                    d0p = (
                        max(0, (kts[0] - kpb * j) * 128) if causal else 0
                    )
                    stps = [
                        psum.tile(
                            [128, 2 * CHUNK], F32, tag="stp", bufs=st_bufs,
                            name=f"stp{h}_{j}_{kp}",
                        )
                        for h in heads
                    ]
                    for i, kt in enumerate(kts):
                        diag = causal and kt >= kpb * j
                        d0 = (kt - kpb * j) * 128 if diag else 0
                        for hh, h in enumerate(heads):
                            qk_q = qs[h // 2]
                            qk_k = ks[h // 2]
                            po = (h % 2) * 64
                            nc.tensor.matmul(
                                stps[hh][:, i * CHUNK + d0 : (i + 1) * CHUNK],
                                qk_k[po : po + 64, kt * 128 : (kt + 1) * 128],
                                qk_q[po : po + 64, j * CHUNK + d0 : (j + 1) * CHUNK],
                                start=True,
                                stop=True,
                                skip_group_check=True,
                            )
                    for hh, h in enumerate(heads):
                        e = pe.tile(
                            [128, 2 * CHUNK], BF16, tag="e", name=f"e{h}_{j}_{kp}"
                        )
                        if d0p:
                            e3 = e[:].rearrange("p (s c) -> p s c", c=CHUNK)
                            s3 = stps[hh][:].rearrange("p (s c) -> p s c", c=CHUNK)
                            nc.scalar.activation(
                                e3[:, :, d0p:],
                                s3[:, :, d0p:],
                                mybir.ActivationFunctionType.Exp,
                                scale=float(HD) ** -0.5,
                            )
                        else:
                            nc.scalar.activation(
                                e[:],
                                stps[hh][:],
                                mybir.ActivationFunctionType.Exp,
                                scale=float(HD) ** -0.5,
                            )
                        for i, kt in enumerate(kts):
                            if causal and kt >= kpb * j:
                                d0 = (kt - kpb * j) * 128
                                w = d0 - d0p + 128
                                nc.gpsimd.affine_select(
                                    out=e[:, i * CHUNK + d0p : i * CHUNK + d0 + 128],
                                    in_=e[:, i * CHUNK + d0p : i * CHUNK + d0 + 128],
                                    compare_op=mybir.AluOpType.is_ge,
                                    fill=0.0,
                                    base=-(d0 - d0p),
                                    pattern=[[1, w]],
                                    channel_multiplier=-1,
                                )
                        pending[h].append((e, kts, d0p))
                        if len(pending[h]) >= 2:
                            flush_avs(hh, h)
                for hh, h in enumerate(heads):
                    flush_avs(hh, h)
                    normalize(j, avs[hh], h)

            def attend_chunk_pair(heads, qs, ks, j):
                # per head, score kt-PAIRS into one 2-bank PSUM tile, ONE exp
                # per pair (amortizes the ~352-cycle ACT pipeline fill), causal
                # masking via -1e9 bias matmuls (keeps Pool out of the chain)
                avs = [
                    psum.tile(
                        [128, CHUNK], F32, tag="av", bufs=av_bufs,
                        name=f"av{h}_{j}",
                    )
                    for h in heads
                ]
                last_kt = kpb * j + kpb - 1 if causal else NKT - 1
                npairs = (last_kt + 1) // 2
                pending = {h: [] for h in heads}

                def flush_avs(hh, h):
                    # AV burst: every exp here completed pairs ago, so these
                    # matmuls never head-of-line-block the in-order PE stream
                    for e, kts_ in pending[h]:
                        for i, kt in enumerate(kts_):
                            nc.tensor.matmul(
                                avs[hh][0:65, :],
                                vps[kt][:, h * (HD + 1) : (h + 1) * (HD + 1)],
                                e[:, i * CHUNK : (i + 1) * CHUNK],
                                start=(kt == 0),
                                stop=(kt == last_kt),
                            )
                    pending[h] = []

                for kp in range(npairs):
                    kts = (2 * kp, 2 * kp + 1)
                    for hh, h in enumerate(heads):
                        qk_q = qs[h // 2]
                        qk_k = ks[h // 2]
                        po = (h % 2) * 64
                        stp = psum.tile(
                            [128, 2 * CHUNK], F32, tag="stp", bufs=st_bufs,
                            name=f"stp{h}_{j}_{kp}",
                        )
                        for i, kt in enumerate(kts):
                            diag = causal and kt >= kpb * j
                            d0 = (kt - kpb * j) * 128 if diag else 0
                            nc.tensor.matmul(
                                stp[:, i * CHUNK + d0 : (i + 1) * CHUNK],
                                qk_k[po : po + 64, kt * 128 : (kt + 1) * 128],
                                qk_q[po : po + 64, j * CHUNK + d0 : (j + 1) * CHUNK],
                                start=True,
                                stop=not diag,
                                skip_group_check=True,
                            )
                            if diag:
                                m = kt - kpb * j
                                nc.tensor.matmul(
                                    stp[:, i * CHUNK : i * CHUNK + d0 + 128],
                                    id128[:],
                                    masks[m][:, 0 : d0 + 128],
                                    start=False,
                                    stop=True,
                                    skip_group_check=True,
                                )
                        e = pe.tile(
                            [128, 2 * CHUNK], BF16, tag="e", name=f"e{h}_{j}_{kp}"
                        )
                        nc.scalar.activation(
                            e[:],
                            stp[:],
                            mybir.ActivationFunctionType.Exp,
                            scale=float(HD) ** -0.5,
                        )
                        pending[h].append((e, kts))
                        if len(pending[h]) >= 2:
                            flush_avs(hh, h)
                for hh, h in enumerate(heads):
                    flush_avs(hh, h)
                    normalize(j, avs[hh], h)

            def attend_chunk(heads, qs, ks, j):
                if pair_exp and mask_pool:
                    return attend_chunk_pair_v2(heads, qs, ks, j)
                if pair_exp:
                    return attend_chunk_pair(heads, qs, ks, j)
                # `group` independent ST->exp->AV chains, interleaved
                avs = [
                    psum.tile(
                        [128, CHUNK], F32, tag="av", bufs=av_bufs,
                        name=f"av{h}_{j}",
                    )
                    for h in heads
                ]
                last_kt = kpb * j + kpb - 1 if causal else NKT - 1
                pending = {h: [] for h in heads}

                def flush_avs(hh, h):
                    # lag-2 AV burst: the exp (and diag Pool fixups) for these
                    # tiles completed tiles ago, so the in-order PE stream
                    # never head-of-line-blocks on a fresh exp
                    for e, kt in pending[h]:
                        nc.tensor.matmul(
                            avs[hh][0:65, :],
                            vps[kt][:, h * (HD + 1) : (h + 1) * (HD + 1)],
                            e[:],
                            start=(kt == 0),
                            stop=(kt == last_kt),
                        )
                    pending[h] = []

                for kt in range(last_kt + 1):
                    diag = causal and kt >= kpb * j
                    d0 = (kt - kpb * j) * 128 if diag else 0
                    for hh, h in enumerate(heads):
                        qk_q = qs[h // 2]
                        qk_k = ks[h // 2]
                        po = (h % 2) * 64
                        st = psum.tile(
                            [128, CHUNK], F32, tag="st", bufs=stb,
                            name=f"st{h}_{j}_{kt}",
                        )
                        nc.tensor.matmul(
                            st[:, d0:CHUNK],
                            qk_k[po : po + 64, kt * 128 : (kt + 1) * 128],
                            qk_q[po : po + 64, j * CHUNK + d0 : (j + 1) * CHUNK],
                            start=True,
                            stop=True,
                        )
                        e = pe.tile(
                            [128, CHUNK], BF16, tag="e", name=f"e{h}_{j}_{kt}"
                        )
                        if d0:
                            nc.gpsimd.memset(e[:, 0:d0], 0.0)
                        nc.scalar.activation(
                            e[:, d0:],
                            st[:, d0:],
                            mybir.ActivationFunctionType.Exp,
                            scale=float(HD) ** -0.5,
                        )
                        if diag:
                            # zero where q < k on the exp'd tile
                            nc.gpsimd.affine_select(
                                out=e[:, d0 : d0 + 128],
                                in_=e[:, d0 : d0 + 128],
                                compare_op=mybir.AluOpType.is_ge,
                                fill=0.0,
                                base=0,
                                pattern=[[1, 128]],
                                channel_multiplier=-1,
                            )
                        pending[h].append((e, kt))
                        if len(pending[h]) >= 2:
                            flush_avs(hh, h)
                for hh, h in enumerate(heads):
                    flush_avs(hh, h)
                    normalize(j, avs[hh], h)

            def project_y(j):
                for e_t in range(C // 128):
                    yp = psum.tile([128, CHUNK], F32, tag="proj", bufs=proj_bufs, name=f"yp{e_t}_{j}")
                    for d in range(DL // 128):
                        nc.tensor.matmul(
                            yp[:],
                            wots[d][:, e_t * 128 : (e_t + 1) * 128],
                            ots[d][:, j * CHUNK : (j + 1) * CHUNK],
                            start=(d == 0),
                            stop=(d == DL // 128 - 1),
                        )
                    ysb = pysb.tile([128, CHUNK], F32, tag="ysb")
                    nc.vector.tensor_copy(ysb[:], yp[:])
                    nc.sync.dma_start(
                        yt_d[e_t * 128 : (e_t + 1) * 128, j * CHUNK : (j + 1) * CHUNK],
                        ysb[:],
                    )

            if sched == "j":
                # chunk-major: all head groups per chunk; V remainder and
                # y-proj(j-1) emitted AFTER chunk j so they fill chunk-j PE gaps
                qs = {pp: qk_tiles[pp] for pp in range(HL // 2)}
                ks = {pp: qk_tiles[NQK // 2 + pp] for pp in range(HL // 2)}
                for j in range(NJ):
                    for g in range(HL // group):
                        heads = [g * group + i for i in range(group)]
                        attend_chunk(heads, qs, ks, j)
                    if j == 0:
                        for ts in range(nv_front, NT_T):
                            vps.append(project_v(ts))
                    if phases >= 3 and j >= 1:
                        project_y(j - 1)
                if phases == 2:
                    for i, ot in enumerate(ots):
                        nc.sync.dma_start(
                            yt_d[i * 128 : (i + 1) * 128, : T // 2].bitcast(BF16), ot[:]
                        )
                    return nc
                project_y(NJ - 1)
            else:
                for g in range(HL // group):
                    heads = [g * group + i for i in range(group)]
                    pps = range(heads[0] // 2, heads[-1] // 2 + 1)
                    qs = {}
                    ks = {}
                    for pp in pps:
                        qs[pp] = qk_tiles.pop(pp, None) or project_qk(pp)
                        ks[pp] = qk_tiles.pop(NQK // 2 + pp, None) or project_qk(NQK // 2 + pp)
                    for j in range(NJ):
                        attend_chunk(heads, qs, ks, j)
                if phases == 2:
                    for i, ot in enumerate(ots):
                        nc.sync.dma_start(
                            yt_d[i * 128 : (i + 1) * 128, : T // 2].bitcast(BF16), ot[:]
                        )
                    return nc
                for j in range(NJ):
                    project_y(j)
    return nc


_CACHE = {}


def _get_nc(T, n_heads_local):
    key = (T, n_heads_local)
    if key not in _CACHE:
        _CACHE[key] = build_nc(T, n_heads_local)
    return _CACHE[key]


def make_in_maps(x, wq, wk, wv, wo):
    x = np.asarray(x, dtype=np.float32)
    wq = np.asarray(wq, dtype=np.float32)
    wk = np.asarray(wk, dtype=np.float32)
    wv = np.asarray(wv, dtype=np.float32)
    wo = np.asarray(wo, dtype=np.float32)
    HL = N_HEADS // 2
    DL = HL * HD
    in_maps = []
    for core in range(8):
        bi = core % 4
        g = core // 4
        gs = slice(g * DL, (g + 1) * DL)
        xt = np.ascontiguousarray(x[bi].T).astype(BF16_NP)
        wqk = np.concatenate([wq[gs].T, wk[gs].T], axis=1).astype(BF16_NP)
        wvt = np.ascontiguousarray(wv[gs].T).astype(BF16_NP)
        wot = np.ascontiguousarray(wo[:, gs].T).astype(BF16_NP)
        in_maps.append(
            {
                "xt": np.ascontiguousarray(xt),
                "wqk": np.ascontiguousarray(wqk),
                "wv": wvt,
                "wot": wot,
            }
        )
    return in_maps


def run(x, wq, wk, wv, wo, trace=False):
    from concourse.bass_utils import run_bass_kernel_spmd

    b, T, C = np.asarray(x).shape
    HL = N_HEADS // 2

    nc = _get_nc(T, HL)
    in_maps = make_in_maps(x, wq, wk, wv, wo)
    try:
        res = run_bass_kernel_spmd(nc, in_maps, list(range(8)), trace=trace)
    except ModuleNotFoundError:
        # NTFF profiling hook unavailable; rerun without tracing
        res = run_bass_kernel_spmd(nc, in_maps, list(range(8)), trace=False)
    y = np.empty((b, T, C), dtype=np.float32)
    for bi in range(b):
        yt = res.results[bi]["yt"] + res.results[bi + 4]["yt"]
        y[bi] = yt.T
    return y, res


def kernel(x, wq, wk, wv, wo):
    y, _ = run(x, wq, wk, wv, wo, trace=False)
    return y

